# revision 66
# baseline (speedup 1.0000x reference)
"""MIND-SSC loss (nn_MindLoss) Trainium2 Bass kernel, v3.

kernel(predict, target) -> np.float32 scalar loss, 8 NeuronCores data-parallel
over depth (16 output planes per core + halo).

v3 restructure vs v2: squared differences are stored fp8e4m3 (validated
+2.8e-4 loss shift in the numpy mirror), which unlocks fp8 DoubleRow matmuls
on PE (0.5 cyc/row, pairing two (dz, w-shift) box-blur taps per instruction).
The 27-tap blur per z-plane drops from 18 bf16 matmuls to 15 (3 dz-pairs + 1
w-pair DR + 1 normal per 4-ch group), t_t (DVE W-pair pre-sum) is gone, and
the channel SUM (v2's 203 us Pool tree) moves to PE: 6 identity-pair DR
matmuls per sq slot accumulate sum_c sq_c into PSUM, evac'd fp8, then 5 more
matmuls blur it into ub = sum_c ssd_c per z.  mv = ub/12 - min.

Per (n, tensor) pass, per core:
  diffs + W-edge replication on the diff cols (DVE, bf16) -> square
  (ACT/DVE/Pool per SQ_PLAN; fp8 out) -> chansum (PE, 6 DR/slot -> u_ps;
  ACT evac fp8) -> per z: 15 main DR/normal matmuls + 5 u-blur matmuls ->
  ACT evac psum->bf16 sb -> per 2-z group: min tree (DVE) -> minsub (DVE)
  -> ub evac (ACT bf16) -> mv STT -> 1/mv (DVE fast recip) -> d *= ninv
  (DVE) -> exp (ACT; p-passes write bf16 e_p, single-buffered -- the
  pass order p,t,p,t makes the cross-batch WAR free); t-passes:
  e_p - e_t (Pool/DVE split; all-DVE on the last pass, whose endgame has
  DVE idle) then (.)^2 accum (ACT Square accum_out).  Host sums the 8
  per-core [H, 32] partials.

Issue-order notes (all engines in-order): chansum(b) is issued AFTER the
emits that need only blocks <= b-1, so PE never queues ready matmuls
behind a slow square; tails are skewed (6,4,10) emit-slots behind their
data; per-z ACT quanta keep PSUM-freeing evacs from queueing behind long
ops; the last pass's exp/loss-square DO batch per-group (BSQA/BEXP in the
UB2 tail path) since nothing trails them.  PSUM banks: main 3x2 + u_ps 1 +
ub 1 = 8 exactly.  Tunables (MIND_* env) were fixed by TimelineSim search.

D-edge replication is baked per-core into the taps input (rows 1-3); H-edge
into the tridiagonal A (corner 2s); W-edge by replicating diff cols 0/129.
ssd/mv are the unscaled 27-tap box sums (exp(-mind/mv) is scale-invariant).

Lone (dz2,s1) taps run as (A,0)-stationary DoubleRows over a stride-0
moving pair (row 5/6 of taps), halving their PE cycles.

v2 baseline: 395411 ns.  v3: 312616 ns (TimelineSim), rel err ~1.8e-3.
"""

import os
import numpy as np
import ml_dtypes

N = 2            # batch
DVOL = 128       # global depth
H = 128
W = 128
CH = 12
NCORES = 8
NZ = DVOL // NCORES       # output planes per core
WP = W + 6                # padded width (3 each side)
WD = W + 2                # diff/sq width (w in [-1 .. 128])
ZB = 3                    # z'-block size
ZG = int(os.environ.get("MIND_ZG", "2"))  # z-group size for tail stages
TOTAL_COUNT = N * CH * DVOL * H * W

BF16 = ml_dtypes.bfloat16
FP8 = ml_dtypes.float8_e4m3


def _blur_matrix():
    A = np.zeros((H, H), np.float32)
    for i in range(H):
        for dh in (-1, 0, 1):
            A[i, min(max(i + dh, 0), H - 1)] += 1.0
    return A


def build_bass(nz=NZ):
    import concourse.bacc as bacc
    import concourse.bass as bass
    import concourse.mybir as mybir
    from concourse.tile import TileContext

    Op = mybir.AluOpType
    Act = mybir.ActivationFunctionType
    DR = mybir.MatmulPerfMode.DoubleRow
    dt = mybir.dt

    ns = nz + 6               # img slots
    nsq = nz + 2              # sq slots
    assert nsq % ZB == 0
    nblk = nsq // ZB
    zg = min(ZG, nz)
    n_zg = nz // zg
    nslot = N * n_zg

    # ---- tunables ----
    SKEW_A1, SKEW_A2, SKEW_B = (
        int(os.environ.get("MIND_SK0", "6")),
        int(os.environ.get("MIND_SK1", "4")),
        int(os.environ.get("MIND_SK2", "10")))
    # per-block square engine: 'a' ACT, 'd' DVE, 'p' Pool
    SQ_PLAN = os.environ.get("MIND_SQPLAN", "apapdp")
    LS_POOL = bool(int(os.environ.get("MIND_LSPOOL", "1")))
    PF_B = int(os.environ.get("MIND_PFB", "5"))
    BUFS_D = int(os.environ.get("MIND_BUFSD", "5"))
    BUFS_Q = int(os.environ.get("MIND_BUFSQ", "5"))
    BUFS_ST = int(os.environ.get("MIND_BUFSST", "6"))
    BUFS_PS = int(os.environ.get("MIND_BUFSPS", "2"))
    BUFS_PU = int(os.environ.get("MIND_BUFSPU", "1"))
    BUFS_PB = int(os.environ.get("MIND_BUFSPB", "1"))
    BEXP = bool(int(os.environ.get("MIND_BEXP", "1")))      # batch exp per group
    BSQA = bool(int(os.environ.get("MIND_BSQA", "1")))      # batch loss-square
    UBEVAC = bool(int(os.environ.get("MIND_UBEVAC", "1")))  # evac ub psum->bf16
    RECBF = bool(int(os.environ.get("MIND_RECBF", "0")))    # recip writes bf16 directly
    SQQ_A = bool(int(os.environ.get("MIND_SQQA", "0")))     # ACT squares per-jj
    SQQ_D = bool(int(os.environ.get("MIND_SQQD", "0")))     # DVE squares per-jj
    SQQ_P = bool(int(os.environ.get("MIND_SQQP", "0")))     # Pool squares per-half
    UEVD = bool(int(os.environ.get("MIND_UEVD", "0")))      # u_ps evac on DVE
    LSQ_TTR = bool(int(os.environ.get("MIND_LSQTTR", "0")))  # last-pass lsq via DVE TTR
    SQ_PLAN0 = os.environ.get("MIND_SQPLAN0", "aadapp")      # pass-0 square plan
    LS_LAST = os.environ.get("MIND_LSLAST", "d")            # last-pass ls engine
    LS_MID = os.environ.get("MIND_LSMID", "m")              # other t-pass ls engine
    M6_POOL = bool(int(os.environ.get("MIND_M6POOL", "0")))  # m6 min stage on Pool
    STT_POOL = bool(int(os.environ.get("MIND_STTPOOL", "0")))  # mv STT on Pool
    NINV_POOL = bool(int(os.environ.get("MIND_NINVPOOL", "0")))  # ninv copy on Pool
    PREFILL3 = bool(int(os.environ.get("MIND_PREFILL3", "0")))   # prefill 3 blocks
    FAST0 = bool(int(os.environ.get("MIND_FAST0", "0")))         # block-0 fast DMA
    EPBF = bool(int(os.environ.get("MIND_EPBF", "1")))           # e_p bf16, 1-buf
    SQ_PLANL = os.environ.get("MIND_SQPLANL", "npnpnp")      # last-pass square plan
    SQ_PLAN1 = os.environ.get("MIND_SQPLAN1", SQ_PLAN)      # pass-1 square plan
    SQ_PLAN2 = os.environ.get("MIND_SQPLAN2", SQ_PLAN)      # pass-2 square plan
    DMA2 = bool(int(os.environ.get("MIND_DMA2", "0")))   # xh on a 2nd DMA ring
    ZPAIR = bool(int(os.environ.get("MIND_ZP", "1")))    # lone matmuls as (A,0) DRs
    UB2 = bool(int(os.environ.get("MIND_UB2", "1")))     # ub psum spans 2 groups
    PF2 = bool(int(os.environ.get("MIND_PF2", "0")))     # prefill 2 blocks at PF_B
    REPLP = bool(int(os.environ.get("MIND_REPLP", "0")))  # W-repl on Pool
    SKL = tuple(int(x) for x in
                os.environ.get("MIND_SKL", "10,11,12")
                .replace(";", ",").split(","))              # last-pass skews
    SKP0 = tuple(int(x) for x in
                 os.environ.get("MIND_SKP0", "5,6,10")
                 .replace(";", ",").split(","))             # pass-0 skews
    SKP1 = tuple(int(x) for x in
                 os.environ.get("MIND_SKP1", "").replace(";", ",").split(",")) \
        if os.environ.get("MIND_SKP1") else None            # pass-1 skews
    SKP2 = tuple(int(x) for x in
                 os.environ.get("MIND_SKP2", "").replace(";", ",").split(",")) \
        if os.environ.get("MIND_SKP2") else None            # pass-2 skews

    CHWD = CH * WD            # slot stride inside an sq block tile
    PB = ZB * CHWD            # partition stride of sq block tile
    UPB = ZB * WD             # partition stride of u block tile

    nc = bacc.Bacc("TRN2", name="mindloss3", target_bir_lowering=False)

    imgs, xhps = {}, {}
    for t in ("p", "t"):
        imgs[t] = nc.dram_tensor(f"img_{t}", [N, H, ns, WP], dt.bfloat16,
                                 kind="ExternalInput")
        xhps[t] = nc.dram_tensor(f"xh_{t}", [N, 2, nsq, H, WP], dt.bfloat16,
                                 kind="ExternalInput")
    taps_d = nc.dram_tensor("taps", [H, 7, 2, H], dt.float8e4,
                            kind="ExternalInput")
    out_stats = nc.dram_tensor("out_stats", [H, nslot * zg], dt.float32,
                               kind="ExternalOutput")

    with TileContext(nc) as tc:
        with tc.tile_pool(name="const", bufs=1) as cpool, \
             tc.tile_pool(name="imgp", bufs=2) as ipool, \
             tc.tile_pool(name="diffp", bufs=BUFS_D) as dpool, \
             tc.tile_pool(name="sqp", bufs=BUFS_Q) as qpool, \
             tc.tile_pool(name="up", bufs=BUFS_Q) as upool, \
             tc.tile_pool(name="stage", bufs=BUFS_ST) as stpool, \
             tc.tile_pool(name="tailp", bufs=2) as tpool, \
             tc.tile_pool(name="tail1", bufs=2) as tpool1, \
             tc.tile_pool(name="epp", bufs=(1 if EPBF else 2)) as eppool, \
             tc.tile_pool(name="psumb", bufs=BUFS_PS, space="PSUM") as ppool, \
             tc.tile_pool(name="psumu", bufs=BUFS_PU, space="PSUM") as pupool, \
             tc.tile_pool(name="psumub", bufs=BUFS_PB, space="PSUM") as pbpool:

            passes = [(n_, t_) for n_ in range(N) for t_ in ("p", "t")]
            loaded = {}

            fast0 = [None]

            def load_pass(idx):
                if idx >= len(passes) or idx in loaded:
                    return
                n_, t_ = passes[idx]
                if idx == 0 and FAST0:
                    x0 = cpool.tile([H, 7, WP], dt.bfloat16, name="x0_t")
                    xh0 = cpool.tile([H, 2, 3, WP], dt.bfloat16, name="xh0_t")
                    nc.sync.dma_start(out=x0[:], in_=imgs[t_][n_][:, 0:7, :])
                    for v in range(2):
                        nc.sync.dma_start(
                            out=xh0[:, v, :, :],
                            in_=xhps[t_][n_][v, 0:3, :, :]
                            .rearrange("j h w -> h j w"))
                    fast0[0] = (x0, xh0)
                xt = ipool.tile([H, ns, WP], dt.bfloat16, tag="x", name="x_t")
                xht = ipool.tile([H, 2, nsq, WP], dt.bfloat16, tag="xh",
                                 name="xh_t")
                nc.sync.dma_start(out=xt[:], in_=imgs[t_][n_])
                if DMA2:
                    nc.gpsimd.dma_start(out=xht[:], in_=xhps[t_][n_])
                else:
                    nc.sync.dma_start(out=xht[:], in_=xhps[t_][n_])
                loaded[idx] = (xt, xht)

            load_pass(0)

            # ACT table warmup (see v2 docstring)
            warm = cpool.tile([1, 1], dt.float32, name="warm")
            nc.vector.memset(warm[:], 0.0)
            nc.scalar.activation(warm[:], warm[:], Act.Exp)
            nc.scalar.activation(warm[:], warm[:], Act.Square)

            taps_t = cpool.tile([H, 7, 2, H], dt.float8e4, name="taps_t")
            nc.sync.dma_start(out=taps_t[:], in_=taps_d[:])

            loss_acc = cpool.tile([H, nslot * zg], dt.float32, name="loss_acc")

            pend = []
            gslot = [0]

            def make_pass(pidx, n, t, e_p, skews=None, ls_eng="p"):
                    x_t, xh_t = loaded[pidx]

                    use0 = [False]

                    def xview(j0, s0_rel, col0, colstep):
                        if use0[0]:
                            x0 = fast0[0][0]
                            return bass.AP(
                                x0[:].tensor, (j0 + s0_rel) * WP + col0,
                                [[7 * WP, H], [WP, ZB], [colstep, 2], [1, WD]])
                        return bass.AP(
                            x_t[:].tensor, (j0 + s0_rel) * WP + col0,
                            [[ns * WP, H], [WP, ZB], [colstep, 2], [1, WD]])

                    def xhview(j0, v0, vstep):
                        if use0[0]:
                            xh0 = fast0[0][1]
                            return bass.AP(
                                xh0[:].tensor, v0 * 3 * WP + j0 * WP + 2,
                                [[2 * 3 * WP, H], [WP, ZB],
                                 [vstep * 3 * WP, 2], [1, WD]])
                        return bass.AP(
                            xh_t[:].tensor,
                            v0 * nsq * WP + j0 * WP + 2,
                            [[2 * nsq * WP, H], [WP, ZB],
                             [vstep * nsq * WP, 2], [1, WD]])

                    def dgroups(j0):
                        return [
                            (0, 3, xview(j0, 2, 0, 4), xview(j0, 0, 2, 0)),
                            (5, 2, xview(j0, 4, 2, 0), xview(j0, 2, 0, 4)),
                            (1, 7, xhview(j0, 1, -1), xview(j0, 0, 2, 0)),
                            (2, 2, xhview(j0, 1, 0), xview(j0, 2, 0, 4)),
                            (6, 5, xview(j0, 4, 2, 0), xhview(j0, 1, -1)),
                            (9, 1, xhview(j0, 0, 0), xview(j0, 2, 0, 4)),
                        ]

                    diff_blocks = {}
                    sq_blocks = {}
                    u_blocks = {}
                    groups = {}
                    emitted = [0]
                    stage_d = None
                    ub_cur = [None]

                    def do_diffs(b):
                        use0[0] = (pidx == 0 and b == 0 and FAST0
                                   and fast0[0] is not None)
                        j0 = b * ZB
                        d_t = dpool.tile([H, ZB, CH, WD], dt.bfloat16,
                                         tag="d", name="d_t")
                        for ch0, chstep, in0, in1 in dgroups(j0):
                            out_ap = bass.AP(
                                d_t[:].tensor, ch0 * WD,
                                [[ZB * CH * WD, H], [CH * WD, ZB],
                                 [chstep * WD, 2], [1, WD]])
                            nc.vector.tensor_tensor(out_ap, in0, in1, Op.subtract)
                        use0[0] = False
                        # W-edge replication on the diffs (sq inherits it):
                        # col0 <- col1, col129 <- col128
                        eo = bass.AP(d_t[:].tensor, 0,
                                     [[PB, H], [CHWD, ZB], [WD, CH], [WD - 1, 2]])
                        ei = bass.AP(d_t[:].tensor, 1,
                                     [[PB, H], [CHWD, ZB], [WD, CH], [WD - 3, 2]])
                        if REPLP:
                            nc.gpsimd.tensor_copy(eo, ei)
                        else:
                            nc.vector.tensor_copy(eo, ei)
                        diff_blocks[b] = d_t

                    def do_square(b):
                        d_t = diff_blocks[b]
                        sq_t = qpool.tile([H, ZB, CH, WD], dt.float8e4,
                                          tag="q", name="sq_t")
                        plan = (SQ_PLAN0 if pidx == 0 else
                                SQ_PLAN1 if pidx == 1 else
                                SQ_PLAN2 if pidx == 2 else SQ_PLANL)
                        eng = plan[b % len(plan)]
                        if eng == 'm':      # jj 0,1 on ACT; jj2 on DVE
                            nc.scalar.square(sq_t[:, 0:2], d_t[:, 0:2])
                            nc.vector.tensor_tensor(sq_t[:, 2:3], d_t[:, 2:3],
                                                    d_t[:, 2:3], Op.mult)
                        elif eng == 'n':    # jj 0,1 on ACT; jj2 on Pool
                            nc.scalar.square(sq_t[:, 0:2], d_t[:, 0:2])
                            nc.gpsimd.tensor_tensor(sq_t[:, 2:3], d_t[:, 2:3],
                                                    d_t[:, 2:3], Op.mult)
                        elif eng == 'q':    # jj0 ACT, jj 1,2 Pool
                            nc.scalar.square(sq_t[:, 0:1], d_t[:, 0:1])
                            for jj in (1, 2):
                                nc.gpsimd.tensor_tensor(
                                    sq_t[:, jj:jj + 1], d_t[:, jj:jj + 1],
                                    d_t[:, jj:jj + 1], Op.mult)
                        elif eng == 'a':
                            if SQQ_A:
                                for jj in range(ZB):
                                    nc.scalar.square(sq_t[:, jj:jj + 1],
                                                     d_t[:, jj:jj + 1])
                            else:
                                nc.scalar.square(sq_t[:], d_t[:])
                        elif eng == 'd':
                            if SQQ_D:
                                for jj in range(ZB):
                                    nc.vector.tensor_tensor(
                                        sq_t[:, jj:jj + 1], d_t[:, jj:jj + 1],
                                        d_t[:, jj:jj + 1], Op.mult)
                            else:
                                nc.vector.tensor_tensor(sq_t[:], d_t[:], d_t[:],
                                                        Op.mult)
                        else:
                            for jj in range(ZB):
                                if SQQ_P:
                                    for hh in range(2):
                                        nc.gpsimd.tensor_tensor(
                                            sq_t[:, jj:jj + 1, 6 * hh:6 * hh + 6],
                                            d_t[:, jj:jj + 1, 6 * hh:6 * hh + 6],
                                            d_t[:, jj:jj + 1, 6 * hh:6 * hh + 6],
                                            Op.mult)
                                else:
                                    nc.gpsimd.tensor_tensor(
                                        sq_t[:, jj:jj + 1], d_t[:, jj:jj + 1],
                                        d_t[:, jj:jj + 1], Op.mult)
                        sq_blocks[b] = sq_t

                    def do_chansum(b):
                        sq_t = sq_blocks[b]
                        T = sq_t[:].tensor
                        u_ps = pupool.tile([H, ZB, WD], dt.float32, tag="ups",
                                           name="u_ps")
                        for jj in range(ZB):
                            for p in range(6):
                                mvu = bass.AP(T, jj * CHWD + p * 2 * WD,
                                              [[PB, H], [WD, 2], [1, WD]])
                                nc.tensor.matmul(u_ps[:, jj, :],
                                                 taps_t[:, 4, :, :], mvu,
                                                 start=(p == 0), stop=(p == 5),
                                                 perf_mode=DR)
                        u_t = upool.tile([H, ZB, WD], dt.float8e4, tag="u",
                                         name="u_t")
                        if UEVD:
                            nc.vector.tensor_copy(u_t[:], u_ps[:])
                        else:
                            nc.scalar.copy(u_t[:], u_ps[:])
                        u_blocks[b] = u_t

                    def emit_z(zi):
                        psum_t = ppool.tile([H, CH, W], dt.float32, tag="ps",
                                            name="psum_t")
                        # slots zi, zi+1, zi+2; consecutive duo in one block
                        if zi % ZB < 2:
                            bP, jP = zi // ZB, zi % ZB        # pair = (dz0, dz1)
                            bL, jL = (zi + 2) // ZB, (zi + 2) % ZB  # lone dz2
                            pair_row = 1 if zi == 0 else (2 if zi == nz - 1 else 0)
                            wl_row = 3 if zi == nz - 1 else 0
                        else:
                            bP, jP = (zi + 1) // ZB, 0        # pair = (dz1, dz2)
                            bL, jL = zi // ZB, 2              # lone dz0
                            pair_row = 0
                            wl_row = 0
                        TP = sq_blocks[bP][:].tensor
                        TL = sq_blocks[bL][:].tensor
                        for g in range(3):
                            c0 = 4 * g * WD
                            out = psum_t[:, 4 * g:4 * g + 4, :]
                            for s in range(3):
                                mv = bass.AP(TP, jP * CHWD + c0 + s,
                                             [[PB, H], [CHWD, 2], [WD, 4], [1, W]])
                                nc.tensor.matmul(out, taps_t[:, pair_row, :, :],
                                                 mv, start=(s == 0), stop=False,
                                                 perf_mode=DR)
                            mvw = bass.AP(TL, jL * CHWD + c0,
                                          [[PB, H], [2, 2], [WD, 4], [1, W]])
                            nc.tensor.matmul(out, taps_t[:, wl_row, :, :], mvw,
                                             start=False, stop=False,
                                             perf_mode=DR)
                            if ZPAIR:
                                zrow = 6 if wl_row == 3 else 5
                                mvl = bass.AP(TL, jL * CHWD + c0 + 1,
                                              [[PB, H], [0, 2], [WD, 4], [1, W]])
                                nc.tensor.matmul(out, taps_t[:, zrow, :, :],
                                                 mvl, start=False, stop=True,
                                                 perf_mode=DR)
                            else:
                                mvl = bass.AP(TL, jL * CHWD + c0 + 1,
                                              [[PB, H], [WD, 4], [1, W]])
                                nc.tensor.matmul(out, taps_t[:, wl_row, 0, :],
                                                 mvl, start=False, stop=True)
                        # u-blur into ub[:, zi%zg, :]
                        UP = u_blocks[bP][:].tensor
                        UL = u_blocks[bL][:].tensor
                        ub = ub_cur[0]
                        uout = ub[:, zi % (2 * zg) if UB2 else zi % zg, :]
                        for s in range(3):
                            mv = bass.AP(UP, jP * WD + s,
                                         [[UPB, H], [WD, 2], [1, W]])
                            nc.tensor.matmul(uout, taps_t[:, pair_row, :, :],
                                             mv, start=(s == 0), stop=False,
                                             perf_mode=DR)
                        mvw = bass.AP(UL, jL * WD, [[UPB, H], [2, 2], [1, W]])
                        nc.tensor.matmul(uout, taps_t[:, wl_row, :, :], mvw,
                                         start=False, stop=False, perf_mode=DR)
                        if ZPAIR:
                            zrow = 6 if wl_row == 3 else 5
                            mvl = bass.AP(UL, jL * WD + 1,
                                          [[UPB, H], [0, 2], [1, W]])
                            nc.tensor.matmul(uout, taps_t[:, zrow, :, :], mvl,
                                             start=False, stop=True,
                                             perf_mode=DR)
                        else:
                            mvl = bass.AP(UL, jL * WD + 1, [[UPB, H], [1, W]])
                            nc.tensor.matmul(uout, taps_t[:, wl_row, 0, :], mvl,
                                             start=False, stop=True)
                        nc.scalar.copy(stage_d[:, zi % zg, :, :], psum_t[:])

                    def close_group(g0):
                        ub = ub_cur[0]
                        if UB2:
                            # evac only at the close of each 2-group window
                            if (g0 // zg) % 2 == 1:
                                ubs = tpool1.tile([H, 2 * zg, W], dt.bfloat16,
                                                  tag="ubs", name="ubs")
                                nc.scalar.copy(ubs[:], ub[:])
                                groups[g0][1]["ubs"] = ubs[:, zg:2 * zg, :]
                                groups[g0 - zg][1]["ubs"] = ubs[:, 0:zg, :]
                            return
                        if UBEVAC:
                            # evac ub psum -> bf16 so the psum tile frees early
                            ubs = tpool1.tile([H, zg, W], dt.bfloat16, tag="ubs",
                                              name="ubs")
                            nc.scalar.copy(ubs[:], ub[:])
                            groups[g0][1]["ubs"] = ubs
                        else:
                            groups[g0][1]["ubs"] = ub

                    def tail_a1(g0, t_, n_, groups_):
                        sb, tl = groups_[g0]
                        m6 = tpool.tile([H, zg, 6, W], dt.bfloat16, tag="m6",
                                        name="m6")
                        m6_eng = nc.gpsimd if M6_POOL else nc.vector
                        m6_eng.tensor_tensor(m6[:], sb[:, :, 0:6, :],
                                             sb[:, :, 6:12, :], Op.min)
                        m3 = tpool.tile([H, zg, 3, W], dt.bfloat16, tag="m3",
                                        name="m3")
                        nc.vector.tensor_tensor(m3[:], m6[:, :, 0:3, :],
                                                m6[:, :, 3:6, :], Op.min)
                        minv = tpool.tile([H, zg, 1, W], dt.bfloat16, tag="minv",
                                          name="minv")
                        nc.vector.tensor_tensor(minv[:], m3[:, :, 0:1, :],
                                                m3[:, :, 1:2, :], Op.min)
                        nc.vector.tensor_tensor(minv[:], minv[:],
                                                m3[:, :, 2:3, :], Op.min)
                        minb = minv[:].broadcast_to([H, zg, CH, W])
                        nc.vector.tensor_tensor(sb, sb, minb, Op.subtract)
                        tl["minv"] = minv

                    def tail_a2(g0, t_, n_, groups_):
                        sb, tl = groups_[g0]
                        minv = tl["minv"]
                        ubs = tl["ubs"]
                        if UB2:
                            mv_f = tpool1.tile([H, zg, W], dt.float32, tag="mvf",
                                               name="mv_f")
                            stt_eng2 = nc.gpsimd if STT_POOL else nc.vector
                            stt_eng2.scalar_tensor_tensor(
                                mv_f[:].unsqueeze(2), ubs.unsqueeze(2),
                                1.0 / 12.0, minv[:], Op.mult, Op.subtract)
                            ninf = tpool1.tile([H, zg, W], dt.float32,
                                               tag="ninf", name="ninf")
                            nc.vector.reciprocal_approx_fast(ninf[:], mv_f[:])
                            ninv = tpool1.tile([H, zg, 1, W], dt.bfloat16,
                                               tag="ninv", name="ninv")
                            nc.vector.tensor_copy(ninv[:], ninf[:].unsqueeze(2))
                            ninvb = ninv[:].broadcast_to([H, zg, CH, W])
                            nc.vector.tensor_tensor(sb, sb, ninvb, Op.mult)
                            if BEXP:
                                if t_ == "p":
                                    nc.scalar.activation(
                                        e_p[:, g0:g0 + zg, :, :], sb,
                                        Act.Exp, scale=-1.0)
                                else:
                                    nc.scalar.activation(sb, sb, Act.Exp,
                                                         scale=-1.0)
                            else:
                                for q in range(zg):
                                    if t_ == "p":
                                        nc.scalar.activation(
                                            e_p[:, g0 + q:g0 + q + 1, :, :],
                                            sb[:, q:q + 1, :, :], Act.Exp,
                                            scale=-1.0)
                                    else:
                                        nc.scalar.activation(
                                            sb[:, q:q + 1, :, :],
                                            sb[:, q:q + 1, :, :], Act.Exp,
                                            scale=-1.0)
                            return
                        mv_f = tpool1.tile([H, zg, W], dt.float32, tag="mvf",
                                           name="mv_f")
                        stt_eng = nc.gpsimd if STT_POOL else nc.vector
                        stt_eng.scalar_tensor_tensor(
                            mv_f[:].unsqueeze(2), ubs[:].unsqueeze(2),
                            1.0 / 12.0, minv[:], Op.mult, Op.subtract)
                        if RECBF:
                            ninv = tpool1.tile([H, zg, 1, W], dt.bfloat16,
                                               tag="ninv", name="ninv")
                            nc.vector.reciprocal_approx_fast(
                                ninv[:].squeeze(2), mv_f[:])
                        else:
                            ninf = tpool1.tile([H, zg, W], dt.float32, tag="ninf",
                                               name="ninf")
                            nc.vector.reciprocal_approx_fast(ninf[:], mv_f[:])
                            ninv = tpool1.tile([H, zg, 1, W], dt.bfloat16,
                                               tag="ninv", name="ninv")
                            if NINV_POOL:
                                nc.gpsimd.tensor_copy(ninv[:],
                                                      ninf[:].unsqueeze(2))
                            else:
                                nc.vector.tensor_copy(ninv[:],
                                                      ninf[:].unsqueeze(2))
                        ninvb = ninv[:].broadcast_to([H, zg, CH, W])
                        nc.vector.tensor_tensor(sb, sb, ninvb, Op.mult)
                        if BEXP:
                            if t_ == "p":
                                nc.scalar.activation(
                                    e_p[:, g0:g0 + zg, :, :], sb,
                                    Act.Exp, scale=-1.0)
                            else:
                                nc.scalar.activation(sb, sb, Act.Exp, scale=-1.0)
                        else:
                            for q in range(zg):
                                if t_ == "p":
                                    nc.scalar.activation(
                                        e_p[:, g0 + q:g0 + q + 1, :, :],
                                        sb[:, q:q + 1, :, :], Act.Exp, scale=-1.0)
                                else:
                                    nc.scalar.activation(
                                        sb[:, q:q + 1, :, :], sb[:, q:q + 1, :, :],
                                        Act.Exp, scale=-1.0)

                    def tail_b(g0, t_, n_, groups_):
                        sb, tl = groups_[g0]
                        ls_dve = ls_eng == "d"
                        if ls_eng == "m":
                            for q in range(zg):
                                eng = nc.gpsimd if q == 0 else nc.vector
                                eng.tensor_tensor(
                                    sb[:, q:q + 1, :, :],
                                    e_p[:, g0 + q:g0 + q + 1, :, :],
                                    sb[:, q:q + 1, :, :], Op.subtract)
                                slot = (n_ * n_zg + g0 // zg) * zg + q
                                nc.scalar.activation(
                                    sb[:, q:q + 1, :, :], sb[:, q:q + 1, :, :],
                                    Act.Square,
                                    accum_out=loss_acc[:, slot:slot + 1])
                            return
                        sub_eng = nc.vector if ls_dve else nc.gpsimd
                        if ls_dve and LSQ_TTR:
                            for q in range(zg):
                                sub_eng.tensor_tensor(
                                    sb[:, q:q + 1, :, :],
                                    e_p[:, g0 + q:g0 + q + 1, :, :],
                                    sb[:, q:q + 1, :, :], Op.subtract)
                                slot = (n_ * n_zg + g0 // zg) * zg + q
                                nc.vector.tensor_tensor_reduce(
                                    sb[:, q:q + 1, :, :], sb[:, q:q + 1, :, :],
                                    sb[:, q:q + 1, :, :], 1.0, 0.0,
                                    Op.mult, Op.add,
                                    loss_acc[:, slot:slot + 1])
                            return
                        if BSQA:
                            sub_eng.tensor_tensor(
                                sb, e_p[:, g0:g0 + zg, :, :], sb, Op.subtract)
                            slot = (n_ * n_zg + g0 // zg) * zg
                            nc.scalar.activation(
                                sb, sb, Act.Square,
                                accum_out=loss_acc[:, slot:slot + 1])
                        else:
                            for q in range(zg):
                                sub_eng.tensor_tensor(
                                    sb[:, q:q + 1, :, :],
                                    e_p[:, g0 + q:g0 + q + 1, :, :],
                                    sb[:, q:q + 1, :, :], Op.subtract)
                                slot = (n_ * n_zg + g0 // zg) * zg + q
                                nc.scalar.activation(
                                    sb[:, q:q + 1, :, :], sb[:, q:q + 1, :, :],
                                    Act.Square,
                                    accum_out=loss_acc[:, slot:slot + 1])

                    def drain_emits(max_z_excl):
                        nonlocal stage_d
                        while emitted[0] < min(nz, max_z_excl):
                            zi = emitted[0]
                            if zi % zg == 0:
                                stage_d = stpool.tile([H, zg, CH, W], dt.bfloat16,
                                                      tag="stg_d", name="stage_d")
                                groups[zi] = (stage_d[:], {})
                            if (zi % (2 * zg) == 0) if UB2 else (zi % zg == 0):
                                ub_cur[0] = pbpool.tile(
                                    [H, 2 * zg, W] if UB2 else [H, zg, W],
                                    dt.float32,
                                                        tag="ub", name="ub")
                            emit_z(zi)
                            emitted[0] += 1
                            if emitted[0] % zg == 0:
                                g0 = emitted[0] - zg
                                close_group(g0)
                                ctx = (g0, t, n, groups)
                                sk = skews or (SKEW_A1, SKEW_A2, SKEW_B)
                                pend.append([tail_a1, ctx, gslot[0] + sk[0]])
                                pend.append([tail_a2, ctx, gslot[0] + sk[1]])
                                if t == "t":
                                    pend.append([tail_b, ctx, gslot[0] + sk[2]])
                            gslot[0] += 1
                            while pend and pend[0][2] <= gslot[0]:
                                fn_, ctx_, _ = pend.pop(0)
                                fn_(*ctx_)

                    return dict(do_diffs=do_diffs, do_square=do_square,
                                do_chansum=do_chansum, drain=drain_emits,
                                produced=set())

            e_p_cur = [None]
            objs = {}

            def get_obj(k):
                if k >= len(passes) or k in objs:
                    return objs.get(k)
                n_, t_ = passes[k]
                if t_ == "p":
                    e_p_cur[0] = eppool.tile(
                        [H, nz, CH, W],
                        dt.bfloat16 if EPBF else dt.float8e4,
                        tag="ep", name="e_p")
                last = (k == len(passes) - 1)
                objs[k] = make_pass(
                    k, n_, t_, e_p_cur[0],
                    skews=(SKL if last else SKP0 if k == 0 else
                           SKP1 if k == 1 else SKP2),
                    ls_eng=LS_LAST if last else LS_MID)
                return objs[k]

            def produce(o, b):
                if b not in o['produced']:
                    o['do_diffs'](b)
                    o['do_square'](b)
                    o['produced'].add(b)

            load_pass(0)
            for k in range(len(passes)):
                o = get_obj(k)
                for b in range(nblk):
                    produce(o, b)
                    # emits needing blocks <= b-1 go to PE BEFORE chansum(b)
                    # so they don't queue behind square(b)
                    if b >= 1:
                        o['drain'](3 * (b - 1) + 1)
                    o['do_chansum'](b)
                    if b == 2:
                        load_pass(k + 1)
                    nxt = get_obj(k + 1) if b >= PF_B else None
                    if b == PF_B and nxt:
                        produce(nxt, 0)
                        nxt['do_chansum'](0)
                        if PF2:
                            produce(nxt, 1)
                            nxt['do_chansum'](1)
                    if b == PF_B + 1 and nxt:
                        produce(nxt, 1)
                        nxt['do_chansum'](1)
                        if PREFILL3:
                            produce(nxt, 2)
                            nxt['do_chansum'](2)
                o['drain'](nz)
            while pend:
                fn_, ctx_, _ = pend.pop(0)
                fn_(*ctx_)

            nc.sync.dma_start(out=out_stats[:], in_=loss_acc[:])

    nc.compile()
    return nc


def _prep_core(vol, z0, nz):
    """vol: (N, D, H, W) f32 -> (img, xh) bf16 W-padded host-side."""
    D = vol.shape[1]
    ns = nz + 6
    nsq = nz + 2
    idx = np.clip(np.arange(z0 - 3, z0 - 3 + ns), 0, D - 1)
    img = vol[:, idx]
    idxq = np.clip(np.arange(z0 - 1, z0 - 1 + nsq), 0, D - 1)
    base = vol[:, idxq]
    hp = np.clip(np.arange(H) + 2, 0, H - 1)
    hm = np.clip(np.arange(H) - 2, 0, H - 1)
    xh = np.stack([base[:, :, hp, :], base[:, :, hm, :]], axis=1)

    def padw(a):
        return np.pad(a, (((0, 0),) * (a.ndim - 1)) + ((3, 3),),
                      mode='edge').astype(BF16)

    img_t = np.ascontiguousarray(padw(img).transpose(0, 2, 1, 3))
    xh_t = np.ascontiguousarray(padw(xh).transpose(0, 3, 1, 2, 4))
    return img_t, xh_t


def _taps_for_core(first, last):
    """[H, 7, 2, H] fp8: 0=(A,A) 1=z0-pair 2=zL-pair 3=zL-wl 4=(I,I)
    5=(A,Z) 6=zL-lone-pair."""
    A = _blur_matrix()
    Z = np.zeros_like(A)
    I = np.eye(H, dtype=np.float32)
    r0 = np.stack([A, A])
    r1 = np.stack([Z, 2 * A]) if first else r0
    r2 = np.stack([A, 2 * A]) if last else r0
    r3 = np.stack([Z, Z]) if last else r0
    r4 = np.stack([I, I])
    r5 = np.stack([A, Z])
    r6 = np.stack([Z, Z]) if last else r5
    taps = np.stack([r0, r1, r2, r3, r4, r5, r6])  # [7, 2, H(k), H(m)]
    taps = taps.transpose(2, 0, 1, 3)              # [H(k), 5, 2, H(m)]
    return np.ascontiguousarray(taps.astype(FP8))


def make_in_maps(p, t, nz=NZ, ncores=NCORES):
    in_maps = []
    for c in range(ncores):
        z0 = c * nz
        img_p, xh_p = _prep_core(p, z0, nz)
        img_t, xh_t = _prep_core(t, z0, nz)
        in_maps.append({
            "img_p": img_p, "xh_p": xh_p,
            "img_t": img_t, "xh_t": xh_t,
            "taps": _taps_for_core(c == 0, c == ncores - 1),
        })
    return in_maps


LAST_RESULTS = None


def kernel(predict, target):
    global LAST_RESULTS
    from concourse import bass_utils

    p = np.ascontiguousarray(np.asarray(predict)[:, 0])
    t = np.ascontiguousarray(np.asarray(target)[:, 0])

    nc = build_bass()
    in_maps = make_in_maps(p, t)

    trace = bool(int(os.environ.get("MIND_TRACE", "0")))
    res = bass_utils.run_bass_kernel_spmd(
        nc, in_maps, core_ids=list(range(NCORES)), trace=trace)
    LAST_RESULTS = res
    total = sum(float(r["out_stats"].astype(np.float64).sum())
                for r in res.results)
    loss = total / TOTAL_COUNT
    return np.array(loss, dtype=np.float32)


if __name__ == "__main__":
    pred = np.load("/root/problem/inp_p.npy")
    targ = np.load("/root/problem/inp_t.npy")
    print("loss:", kernel(pred, targ))


# revision 67
# speedup vs baseline: 1.0071x; 1.0071x over previous
"""MIND-SSC loss (nn_MindLoss) Trainium2 Bass kernel, v3.

kernel(predict, target) -> np.float32 scalar loss, 8 NeuronCores data-parallel
over depth (16 output planes per core + halo).

v3 restructure vs v2: squared differences are stored fp8e4m3 (validated
+2.8e-4 loss shift in the numpy mirror), which unlocks fp8 DoubleRow matmuls
on PE (0.5 cyc/row, pairing two (dz, w-shift) box-blur taps per instruction).
The 27-tap blur per z-plane drops from 18 bf16 matmuls to 15 (3 dz-pairs + 1
w-pair DR + 1 normal per 4-ch group), t_t (DVE W-pair pre-sum) is gone, and
the channel SUM (v2's 203 us Pool tree) moves to PE: 6 identity-pair DR
matmuls per sq slot accumulate sum_c sq_c into PSUM, evac'd fp8, then 5 more
matmuls blur it into ub = sum_c ssd_c per z.  mv = ub/12 - min.

Per (n, tensor) pass, per core:
  diffs + W-edge replication on the diff cols (DVE, bf16) -> square
  (ACT/DVE/Pool per SQ_PLAN; fp8 out) -> chansum (PE, 6 DR/slot -> u_ps;
  ACT evac fp8) -> per z: 15 main DR/normal matmuls + 5 u-blur matmuls ->
  ACT evac psum->bf16 sb -> per 2-z group: min tree (DVE) -> minsub (DVE)
  -> ub evac (ACT bf16) -> mv STT -> 1/mv (DVE fast recip) -> d *= ninv
  (DVE) -> exp (ACT; p-passes write bf16 e_p, single-buffered -- the
  pass order p,t,p,t makes the cross-batch WAR free); t-passes:
  e_p - e_t (Pool/DVE split; all-DVE on the last pass, whose endgame has
  DVE idle) then (.)^2 accum (ACT Square accum_out).  Host sums the 8
  per-core [H, 32] partials.

Issue-order notes (all engines in-order): chansum(b) is issued AFTER the
emits that need only blocks <= b-1, so PE never queues ready matmuls
behind a slow square; tails are skewed (6,4,10) emit-slots behind their
data; per-z ACT quanta keep PSUM-freeing evacs from queueing behind long
ops; the last pass's exp/loss-square DO batch per-group (BSQA/BEXP in the
UB2 tail path) since nothing trails them.  PSUM banks: main 3x2 + u_ps 1 +
ub 1 = 8 exactly.  Tunables (MIND_* env) were fixed by TimelineSim search.

D-edge replication is baked per-core into the taps input (rows 1-3); H-edge
into the tridiagonal A (corner 2s); W-edge by replicating diff cols 0/129.
ssd/mv are the unscaled 27-tap box sums (exp(-mind/mv) is scale-invariant).

Lone (dz2,s1) taps run as (A,0)-stationary DoubleRows over a stride-0
moving pair (row 5/6 of taps), halving their PE cycles.

v2 baseline: 395411 ns.  v3: 312616 ns (TimelineSim), rel err ~1.8e-3.
"""

import os
import numpy as np
import ml_dtypes

N = 2            # batch
DVOL = 128       # global depth
H = 128
W = 128
CH = 12
NCORES = 8
NZ = DVOL // NCORES       # output planes per core
WP = W + 6                # padded width (3 each side)
WD = W + 2                # diff/sq width (w in [-1 .. 128])
ZB = 3                    # z'-block size
ZG = int(os.environ.get("MIND_ZG", "2"))  # z-group size for tail stages
TOTAL_COUNT = N * CH * DVOL * H * W

BF16 = ml_dtypes.bfloat16
FP8 = ml_dtypes.float8_e4m3


def _blur_matrix():
    A = np.zeros((H, H), np.float32)
    for i in range(H):
        for dh in (-1, 0, 1):
            A[i, min(max(i + dh, 0), H - 1)] += 1.0
    return A


def build_bass(nz=NZ):
    import concourse.bacc as bacc
    import concourse.bass as bass
    import concourse.mybir as mybir
    from concourse.tile import TileContext

    Op = mybir.AluOpType
    Act = mybir.ActivationFunctionType
    DR = mybir.MatmulPerfMode.DoubleRow
    dt = mybir.dt

    ns = nz + 6               # img slots
    nsq = nz + 2              # sq slots
    assert nsq % ZB == 0
    nblk = nsq // ZB
    zg = min(ZG, nz)
    n_zg = nz // zg
    nslot = N * n_zg

    # ---- tunables ----
    SKEW_A1, SKEW_A2, SKEW_B = (
        int(os.environ.get("MIND_SK0", "6")),
        int(os.environ.get("MIND_SK1", "4")),
        int(os.environ.get("MIND_SK2", "10")))
    # per-block square engine: 'a' ACT, 'd' DVE, 'p' Pool
    SQ_PLAN = os.environ.get("MIND_SQPLAN", "apapdp")
    LS_POOL = bool(int(os.environ.get("MIND_LSPOOL", "1")))
    PF_B = int(os.environ.get("MIND_PFB", "5"))
    BUFS_D = int(os.environ.get("MIND_BUFSD", "5"))
    BUFS_Q = int(os.environ.get("MIND_BUFSQ", "5"))
    BUFS_ST = int(os.environ.get("MIND_BUFSST", "6"))
    BUFS_PS = int(os.environ.get("MIND_BUFSPS", "2"))
    BUFS_PU = int(os.environ.get("MIND_BUFSPU", "1"))
    BUFS_PB = int(os.environ.get("MIND_BUFSPB", "1"))
    BEXP = bool(int(os.environ.get("MIND_BEXP", "1")))      # batch exp per group
    BSQA = bool(int(os.environ.get("MIND_BSQA", "1")))      # batch loss-square
    UBEVAC = bool(int(os.environ.get("MIND_UBEVAC", "1")))  # evac ub psum->bf16
    RECBF = bool(int(os.environ.get("MIND_RECBF", "0")))    # recip writes bf16 directly
    SQQ_A = bool(int(os.environ.get("MIND_SQQA", "1")))     # ACT squares per-jj
    SQQ_D = bool(int(os.environ.get("MIND_SQQD", "0")))     # DVE squares per-jj
    SQQ_P = bool(int(os.environ.get("MIND_SQQP", "0")))     # Pool squares per-half
    UEVD = bool(int(os.environ.get("MIND_UEVD", "0")))      # u_ps evac on DVE
    LSQ_TTR = bool(int(os.environ.get("MIND_LSQTTR", "0")))  # last-pass lsq via DVE TTR
    SQ_PLAN0 = os.environ.get("MIND_SQPLAN0", "aadapp")      # pass-0 square plan
    LS_LAST = os.environ.get("MIND_LSLAST", "d")            # last-pass ls engine
    LS_MID = os.environ.get("MIND_LSMID", "m")              # other t-pass ls engine
    M6_POOL = bool(int(os.environ.get("MIND_M6POOL", "0")))  # m6 min stage on Pool
    STT_POOL = bool(int(os.environ.get("MIND_STTPOOL", "0")))  # mv STT on Pool
    NINV_POOL = bool(int(os.environ.get("MIND_NINVPOOL", "0")))  # ninv copy on Pool
    PREFILL3 = bool(int(os.environ.get("MIND_PREFILL3", "0")))   # prefill 3 blocks
    FAST0 = bool(int(os.environ.get("MIND_FAST0", "0")))         # block-0 fast DMA
    EPBF = bool(int(os.environ.get("MIND_EPBF", "1")))           # e_p bf16, 1-buf
    SQ_PLANL = os.environ.get("MIND_SQPLANL", "npnpnp")      # last-pass square plan
    SQ_PLAN1 = os.environ.get("MIND_SQPLAN1", SQ_PLAN)      # pass-1 square plan
    SQ_PLAN2 = os.environ.get("MIND_SQPLAN2", SQ_PLAN)      # pass-2 square plan
    DMA2 = bool(int(os.environ.get("MIND_DMA2", "0")))   # xh on a 2nd DMA ring
    ZPAIR = bool(int(os.environ.get("MIND_ZP", "1")))    # lone matmuls as (A,0) DRs
    UB2 = bool(int(os.environ.get("MIND_UB2", "1")))     # ub psum spans 2 groups
    PF2 = bool(int(os.environ.get("MIND_PF2", "0")))     # prefill 2 blocks at PF_B
    REPLP = bool(int(os.environ.get("MIND_REPLP", "0")))  # W-repl on Pool
    SKL = tuple(int(x) for x in
                os.environ.get("MIND_SKL", "10,11,12")
                .replace(";", ",").split(","))              # last-pass skews
    SKP0 = tuple(int(x) for x in
                 os.environ.get("MIND_SKP0", "5,6,10")
                 .replace(";", ",").split(","))             # pass-0 skews
    SKP1 = tuple(int(x) for x in
                 os.environ.get("MIND_SKP1", "").replace(";", ",").split(",")) \
        if os.environ.get("MIND_SKP1") else None            # pass-1 skews
    SKP2 = tuple(int(x) for x in
                 os.environ.get("MIND_SKP2", "").replace(";", ",").split(",")) \
        if os.environ.get("MIND_SKP2") else None            # pass-2 skews

    CHWD = CH * WD            # slot stride inside an sq block tile
    PB = ZB * CHWD            # partition stride of sq block tile
    UPB = ZB * WD             # partition stride of u block tile

    nc = bacc.Bacc("TRN2", name="mindloss3", target_bir_lowering=False)

    imgs, xhps = {}, {}
    for t in ("p", "t"):
        imgs[t] = nc.dram_tensor(f"img_{t}", [N, H, ns, WP], dt.bfloat16,
                                 kind="ExternalInput")
        xhps[t] = nc.dram_tensor(f"xh_{t}", [N, 2, nsq, H, WP], dt.bfloat16,
                                 kind="ExternalInput")
    taps_d = nc.dram_tensor("taps", [H, 7, 2, H], dt.float8e4,
                            kind="ExternalInput")
    out_stats = nc.dram_tensor("out_stats", [H, nslot * zg], dt.float32,
                               kind="ExternalOutput")

    with TileContext(nc) as tc:
        with tc.tile_pool(name="const", bufs=1) as cpool, \
             tc.tile_pool(name="imgp", bufs=2) as ipool, \
             tc.tile_pool(name="diffp", bufs=BUFS_D) as dpool, \
             tc.tile_pool(name="sqp", bufs=BUFS_Q) as qpool, \
             tc.tile_pool(name="up", bufs=BUFS_Q) as upool, \
             tc.tile_pool(name="stage", bufs=BUFS_ST) as stpool, \
             tc.tile_pool(name="tailp", bufs=2) as tpool, \
             tc.tile_pool(name="tail1", bufs=2) as tpool1, \
             tc.tile_pool(name="epp", bufs=(1 if EPBF else 2)) as eppool, \
             tc.tile_pool(name="psumb", bufs=BUFS_PS, space="PSUM") as ppool, \
             tc.tile_pool(name="psumu", bufs=BUFS_PU, space="PSUM") as pupool, \
             tc.tile_pool(name="psumub", bufs=BUFS_PB, space="PSUM") as pbpool:

            passes = [(n_, t_) for n_ in range(N) for t_ in ("p", "t")]
            loaded = {}

            fast0 = [None]

            def load_pass(idx):
                if idx >= len(passes) or idx in loaded:
                    return
                n_, t_ = passes[idx]
                if idx == 0 and FAST0:
                    x0 = cpool.tile([H, 7, WP], dt.bfloat16, name="x0_t")
                    xh0 = cpool.tile([H, 2, 3, WP], dt.bfloat16, name="xh0_t")
                    nc.sync.dma_start(out=x0[:], in_=imgs[t_][n_][:, 0:7, :])
                    for v in range(2):
                        nc.sync.dma_start(
                            out=xh0[:, v, :, :],
                            in_=xhps[t_][n_][v, 0:3, :, :]
                            .rearrange("j h w -> h j w"))
                    fast0[0] = (x0, xh0)
                xt = ipool.tile([H, ns, WP], dt.bfloat16, tag="x", name="x_t")
                xht = ipool.tile([H, 2, nsq, WP], dt.bfloat16, tag="xh",
                                 name="xh_t")
                nc.sync.dma_start(out=xt[:], in_=imgs[t_][n_])
                if DMA2:
                    nc.gpsimd.dma_start(out=xht[:], in_=xhps[t_][n_])
                else:
                    nc.sync.dma_start(out=xht[:], in_=xhps[t_][n_])
                loaded[idx] = (xt, xht)

            load_pass(0)

            # ACT table warmup (see v2 docstring)
            warm = cpool.tile([1, 1], dt.float32, name="warm")
            nc.vector.memset(warm[:], 0.0)
            nc.scalar.activation(warm[:], warm[:], Act.Exp)
            nc.scalar.activation(warm[:], warm[:], Act.Square)

            taps_t = cpool.tile([H, 7, 2, H], dt.float8e4, name="taps_t")
            nc.sync.dma_start(out=taps_t[:], in_=taps_d[:])

            loss_acc = cpool.tile([H, nslot * zg], dt.float32, name="loss_acc")

            pend = []
            gslot = [0]

            def make_pass(pidx, n, t, e_p, skews=None, ls_eng="p"):
                    x_t, xh_t = loaded[pidx]

                    use0 = [False]

                    def xview(j0, s0_rel, col0, colstep):
                        if use0[0]:
                            x0 = fast0[0][0]
                            return bass.AP(
                                x0[:].tensor, (j0 + s0_rel) * WP + col0,
                                [[7 * WP, H], [WP, ZB], [colstep, 2], [1, WD]])
                        return bass.AP(
                            x_t[:].tensor, (j0 + s0_rel) * WP + col0,
                            [[ns * WP, H], [WP, ZB], [colstep, 2], [1, WD]])

                    def xhview(j0, v0, vstep):
                        if use0[0]:
                            xh0 = fast0[0][1]
                            return bass.AP(
                                xh0[:].tensor, v0 * 3 * WP + j0 * WP + 2,
                                [[2 * 3 * WP, H], [WP, ZB],
                                 [vstep * 3 * WP, 2], [1, WD]])
                        return bass.AP(
                            xh_t[:].tensor,
                            v0 * nsq * WP + j0 * WP + 2,
                            [[2 * nsq * WP, H], [WP, ZB],
                             [vstep * nsq * WP, 2], [1, WD]])

                    def dgroups(j0):
                        return [
                            (0, 3, xview(j0, 2, 0, 4), xview(j0, 0, 2, 0)),
                            (5, 2, xview(j0, 4, 2, 0), xview(j0, 2, 0, 4)),
                            (1, 7, xhview(j0, 1, -1), xview(j0, 0, 2, 0)),
                            (2, 2, xhview(j0, 1, 0), xview(j0, 2, 0, 4)),
                            (6, 5, xview(j0, 4, 2, 0), xhview(j0, 1, -1)),
                            (9, 1, xhview(j0, 0, 0), xview(j0, 2, 0, 4)),
                        ]

                    diff_blocks = {}
                    sq_blocks = {}
                    u_blocks = {}
                    groups = {}
                    emitted = [0]
                    stage_d = None
                    ub_cur = [None]

                    def do_diffs(b):
                        use0[0] = (pidx == 0 and b == 0 and FAST0
                                   and fast0[0] is not None)
                        j0 = b * ZB
                        d_t = dpool.tile([H, ZB, CH, WD], dt.bfloat16,
                                         tag="d", name="d_t")
                        for ch0, chstep, in0, in1 in dgroups(j0):
                            out_ap = bass.AP(
                                d_t[:].tensor, ch0 * WD,
                                [[ZB * CH * WD, H], [CH * WD, ZB],
                                 [chstep * WD, 2], [1, WD]])
                            nc.vector.tensor_tensor(out_ap, in0, in1, Op.subtract)
                        use0[0] = False
                        # W-edge replication on the diffs (sq inherits it):
                        # col0 <- col1, col129 <- col128
                        eo = bass.AP(d_t[:].tensor, 0,
                                     [[PB, H], [CHWD, ZB], [WD, CH], [WD - 1, 2]])
                        ei = bass.AP(d_t[:].tensor, 1,
                                     [[PB, H], [CHWD, ZB], [WD, CH], [WD - 3, 2]])
                        if REPLP:
                            nc.gpsimd.tensor_copy(eo, ei)
                        else:
                            nc.vector.tensor_copy(eo, ei)
                        diff_blocks[b] = d_t

                    def do_square(b):
                        d_t = diff_blocks[b]
                        sq_t = qpool.tile([H, ZB, CH, WD], dt.float8e4,
                                          tag="q", name="sq_t")
                        plan = (SQ_PLAN0 if pidx == 0 else
                                SQ_PLAN1 if pidx == 1 else
                                SQ_PLAN2 if pidx == 2 else SQ_PLANL)
                        eng = plan[b % len(plan)]
                        if eng == 'm':      # jj 0,1 on ACT; jj2 on DVE
                            nc.scalar.square(sq_t[:, 0:2], d_t[:, 0:2])
                            nc.vector.tensor_tensor(sq_t[:, 2:3], d_t[:, 2:3],
                                                    d_t[:, 2:3], Op.mult)
                        elif eng == 'n':    # jj 0,1 on ACT; jj2 on Pool
                            nc.scalar.square(sq_t[:, 0:2], d_t[:, 0:2])
                            nc.gpsimd.tensor_tensor(sq_t[:, 2:3], d_t[:, 2:3],
                                                    d_t[:, 2:3], Op.mult)
                        elif eng == 'q':    # jj0 ACT, jj 1,2 Pool
                            nc.scalar.square(sq_t[:, 0:1], d_t[:, 0:1])
                            for jj in (1, 2):
                                nc.gpsimd.tensor_tensor(
                                    sq_t[:, jj:jj + 1], d_t[:, jj:jj + 1],
                                    d_t[:, jj:jj + 1], Op.mult)
                        elif eng == 'a':
                            if SQQ_A:
                                for jj in range(ZB):
                                    nc.scalar.square(sq_t[:, jj:jj + 1],
                                                     d_t[:, jj:jj + 1])
                            else:
                                nc.scalar.square(sq_t[:], d_t[:])
                        elif eng == 'd':
                            if SQQ_D:
                                for jj in range(ZB):
                                    nc.vector.tensor_tensor(
                                        sq_t[:, jj:jj + 1], d_t[:, jj:jj + 1],
                                        d_t[:, jj:jj + 1], Op.mult)
                            else:
                                nc.vector.tensor_tensor(sq_t[:], d_t[:], d_t[:],
                                                        Op.mult)
                        else:
                            for jj in range(ZB):
                                if SQQ_P:
                                    for hh in range(2):
                                        nc.gpsimd.tensor_tensor(
                                            sq_t[:, jj:jj + 1, 6 * hh:6 * hh + 6],
                                            d_t[:, jj:jj + 1, 6 * hh:6 * hh + 6],
                                            d_t[:, jj:jj + 1, 6 * hh:6 * hh + 6],
                                            Op.mult)
                                else:
                                    nc.gpsimd.tensor_tensor(
                                        sq_t[:, jj:jj + 1], d_t[:, jj:jj + 1],
                                        d_t[:, jj:jj + 1], Op.mult)
                        sq_blocks[b] = sq_t

                    def do_chansum(b):
                        sq_t = sq_blocks[b]
                        T = sq_t[:].tensor
                        u_ps = pupool.tile([H, ZB, WD], dt.float32, tag="ups",
                                           name="u_ps")
                        for jj in range(ZB):
                            for p in range(6):
                                mvu = bass.AP(T, jj * CHWD + p * 2 * WD,
                                              [[PB, H], [WD, 2], [1, WD]])
                                nc.tensor.matmul(u_ps[:, jj, :],
                                                 taps_t[:, 4, :, :], mvu,
                                                 start=(p == 0), stop=(p == 5),
                                                 perf_mode=DR)
                        u_t = upool.tile([H, ZB, WD], dt.float8e4, tag="u",
                                         name="u_t")
                        if UEVD:
                            nc.vector.tensor_copy(u_t[:], u_ps[:])
                        else:
                            nc.scalar.copy(u_t[:], u_ps[:])
                        u_blocks[b] = u_t

                    def emit_z(zi):
                        psum_t = ppool.tile([H, CH, W], dt.float32, tag="ps",
                                            name="psum_t")
                        # slots zi, zi+1, zi+2; consecutive duo in one block
                        if zi % ZB < 2:
                            bP, jP = zi // ZB, zi % ZB        # pair = (dz0, dz1)
                            bL, jL = (zi + 2) // ZB, (zi + 2) % ZB  # lone dz2
                            pair_row = 1 if zi == 0 else (2 if zi == nz - 1 else 0)
                            wl_row = 3 if zi == nz - 1 else 0
                        else:
                            bP, jP = (zi + 1) // ZB, 0        # pair = (dz1, dz2)
                            bL, jL = zi // ZB, 2              # lone dz0
                            pair_row = 0
                            wl_row = 0
                        TP = sq_blocks[bP][:].tensor
                        TL = sq_blocks[bL][:].tensor
                        for g in range(3):
                            c0 = 4 * g * WD
                            out = psum_t[:, 4 * g:4 * g + 4, :]
                            for s in range(3):
                                mv = bass.AP(TP, jP * CHWD + c0 + s,
                                             [[PB, H], [CHWD, 2], [WD, 4], [1, W]])
                                nc.tensor.matmul(out, taps_t[:, pair_row, :, :],
                                                 mv, start=(s == 0), stop=False,
                                                 perf_mode=DR)
                            mvw = bass.AP(TL, jL * CHWD + c0,
                                          [[PB, H], [2, 2], [WD, 4], [1, W]])
                            nc.tensor.matmul(out, taps_t[:, wl_row, :, :], mvw,
                                             start=False, stop=False,
                                             perf_mode=DR)
                            if ZPAIR:
                                zrow = 6 if wl_row == 3 else 5
                                mvl = bass.AP(TL, jL * CHWD + c0 + 1,
                                              [[PB, H], [0, 2], [WD, 4], [1, W]])
                                nc.tensor.matmul(out, taps_t[:, zrow, :, :],
                                                 mvl, start=False, stop=True,
                                                 perf_mode=DR)
                            else:
                                mvl = bass.AP(TL, jL * CHWD + c0 + 1,
                                              [[PB, H], [WD, 4], [1, W]])
                                nc.tensor.matmul(out, taps_t[:, wl_row, 0, :],
                                                 mvl, start=False, stop=True)
                        # u-blur into ub[:, zi%zg, :]
                        UP = u_blocks[bP][:].tensor
                        UL = u_blocks[bL][:].tensor
                        ub = ub_cur[0]
                        uout = ub[:, zi % (2 * zg) if UB2 else zi % zg, :]
                        for s in range(3):
                            mv = bass.AP(UP, jP * WD + s,
                                         [[UPB, H], [WD, 2], [1, W]])
                            nc.tensor.matmul(uout, taps_t[:, pair_row, :, :],
                                             mv, start=(s == 0), stop=False,
                                             perf_mode=DR)
                        mvw = bass.AP(UL, jL * WD, [[UPB, H], [2, 2], [1, W]])
                        nc.tensor.matmul(uout, taps_t[:, wl_row, :, :], mvw,
                                         start=False, stop=False, perf_mode=DR)
                        if ZPAIR:
                            zrow = 6 if wl_row == 3 else 5
                            mvl = bass.AP(UL, jL * WD + 1,
                                          [[UPB, H], [0, 2], [1, W]])
                            nc.tensor.matmul(uout, taps_t[:, zrow, :, :], mvl,
                                             start=False, stop=True,
                                             perf_mode=DR)
                        else:
                            mvl = bass.AP(UL, jL * WD + 1, [[UPB, H], [1, W]])
                            nc.tensor.matmul(uout, taps_t[:, wl_row, 0, :], mvl,
                                             start=False, stop=True)
                        nc.scalar.copy(stage_d[:, zi % zg, :, :], psum_t[:])

                    def close_group(g0):
                        ub = ub_cur[0]
                        if UB2:
                            # evac only at the close of each 2-group window
                            if (g0 // zg) % 2 == 1:
                                ubs = tpool1.tile([H, 2 * zg, W], dt.bfloat16,
                                                  tag="ubs", name="ubs")
                                nc.scalar.copy(ubs[:], ub[:])
                                groups[g0][1]["ubs"] = ubs[:, zg:2 * zg, :]
                                groups[g0 - zg][1]["ubs"] = ubs[:, 0:zg, :]
                            return
                        if UBEVAC:
                            # evac ub psum -> bf16 so the psum tile frees early
                            ubs = tpool1.tile([H, zg, W], dt.bfloat16, tag="ubs",
                                              name="ubs")
                            nc.scalar.copy(ubs[:], ub[:])
                            groups[g0][1]["ubs"] = ubs
                        else:
                            groups[g0][1]["ubs"] = ub

                    def tail_a1(g0, t_, n_, groups_):
                        sb, tl = groups_[g0]
                        m6 = tpool.tile([H, zg, 6, W], dt.bfloat16, tag="m6",
                                        name="m6")
                        m6_eng = nc.gpsimd if M6_POOL else nc.vector
                        m6_eng.tensor_tensor(m6[:], sb[:, :, 0:6, :],
                                             sb[:, :, 6:12, :], Op.min)
                        m3 = tpool.tile([H, zg, 3, W], dt.bfloat16, tag="m3",
                                        name="m3")
                        nc.vector.tensor_tensor(m3[:], m6[:, :, 0:3, :],
                                                m6[:, :, 3:6, :], Op.min)
                        minv = tpool.tile([H, zg, 1, W], dt.bfloat16, tag="minv",
                                          name="minv")
                        nc.vector.tensor_tensor(minv[:], m3[:, :, 0:1, :],
                                                m3[:, :, 1:2, :], Op.min)
                        nc.vector.tensor_tensor(minv[:], minv[:],
                                                m3[:, :, 2:3, :], Op.min)
                        minb = minv[:].broadcast_to([H, zg, CH, W])
                        nc.vector.tensor_tensor(sb, sb, minb, Op.subtract)
                        tl["minv"] = minv

                    def tail_a2(g0, t_, n_, groups_):
                        sb, tl = groups_[g0]
                        minv = tl["minv"]
                        ubs = tl["ubs"]
                        if UB2:
                            mv_f = tpool1.tile([H, zg, W], dt.float32, tag="mvf",
                                               name="mv_f")
                            stt_eng2 = nc.gpsimd if STT_POOL else nc.vector
                            stt_eng2.scalar_tensor_tensor(
                                mv_f[:].unsqueeze(2), ubs.unsqueeze(2),
                                1.0 / 12.0, minv[:], Op.mult, Op.subtract)
                            ninf = tpool1.tile([H, zg, W], dt.float32,
                                               tag="ninf", name="ninf")
                            nc.vector.reciprocal_approx_fast(ninf[:], mv_f[:])
                            ninv = tpool1.tile([H, zg, 1, W], dt.bfloat16,
                                               tag="ninv", name="ninv")
                            nc.vector.tensor_copy(ninv[:], ninf[:].unsqueeze(2))
                            ninvb = ninv[:].broadcast_to([H, zg, CH, W])
                            nc.vector.tensor_tensor(sb, sb, ninvb, Op.mult)
                            if BEXP:
                                if t_ == "p":
                                    nc.scalar.activation(
                                        e_p[:, g0:g0 + zg, :, :], sb,
                                        Act.Exp, scale=-1.0)
                                else:
                                    nc.scalar.activation(sb, sb, Act.Exp,
                                                         scale=-1.0)
                            else:
                                for q in range(zg):
                                    if t_ == "p":
                                        nc.scalar.activation(
                                            e_p[:, g0 + q:g0 + q + 1, :, :],
                                            sb[:, q:q + 1, :, :], Act.Exp,
                                            scale=-1.0)
                                    else:
                                        nc.scalar.activation(
                                            sb[:, q:q + 1, :, :],
                                            sb[:, q:q + 1, :, :], Act.Exp,
                                            scale=-1.0)
                            return
                        mv_f = tpool1.tile([H, zg, W], dt.float32, tag="mvf",
                                           name="mv_f")
                        stt_eng = nc.gpsimd if STT_POOL else nc.vector
                        stt_eng.scalar_tensor_tensor(
                            mv_f[:].unsqueeze(2), ubs[:].unsqueeze(2),
                            1.0 / 12.0, minv[:], Op.mult, Op.subtract)
                        if RECBF:
                            ninv = tpool1.tile([H, zg, 1, W], dt.bfloat16,
                                               tag="ninv", name="ninv")
                            nc.vector.reciprocal_approx_fast(
                                ninv[:].squeeze(2), mv_f[:])
                        else:
                            ninf = tpool1.tile([H, zg, W], dt.float32, tag="ninf",
                                               name="ninf")
                            nc.vector.reciprocal_approx_fast(ninf[:], mv_f[:])
                            ninv = tpool1.tile([H, zg, 1, W], dt.bfloat16,
                                               tag="ninv", name="ninv")
                            if NINV_POOL:
                                nc.gpsimd.tensor_copy(ninv[:],
                                                      ninf[:].unsqueeze(2))
                            else:
                                nc.vector.tensor_copy(ninv[:],
                                                      ninf[:].unsqueeze(2))
                        ninvb = ninv[:].broadcast_to([H, zg, CH, W])
                        nc.vector.tensor_tensor(sb, sb, ninvb, Op.mult)
                        if BEXP:
                            if t_ == "p":
                                nc.scalar.activation(
                                    e_p[:, g0:g0 + zg, :, :], sb,
                                    Act.Exp, scale=-1.0)
                            else:
                                nc.scalar.activation(sb, sb, Act.Exp, scale=-1.0)
                        else:
                            for q in range(zg):
                                if t_ == "p":
                                    nc.scalar.activation(
                                        e_p[:, g0 + q:g0 + q + 1, :, :],
                                        sb[:, q:q + 1, :, :], Act.Exp, scale=-1.0)
                                else:
                                    nc.scalar.activation(
                                        sb[:, q:q + 1, :, :], sb[:, q:q + 1, :, :],
                                        Act.Exp, scale=-1.0)

                    def tail_b(g0, t_, n_, groups_):
                        sb, tl = groups_[g0]
                        ls_dve = ls_eng == "d"
                        if ls_eng == "m":
                            for q in range(zg):
                                eng = nc.gpsimd if q == 0 else nc.vector
                                eng.tensor_tensor(
                                    sb[:, q:q + 1, :, :],
                                    e_p[:, g0 + q:g0 + q + 1, :, :],
                                    sb[:, q:q + 1, :, :], Op.subtract)
                                slot = (n_ * n_zg + g0 // zg) * zg + q
                                nc.scalar.activation(
                                    sb[:, q:q + 1, :, :], sb[:, q:q + 1, :, :],
                                    Act.Square,
                                    accum_out=loss_acc[:, slot:slot + 1])
                            return
                        sub_eng = nc.vector if ls_dve else nc.gpsimd
                        if ls_dve and LSQ_TTR:
                            for q in range(zg):
                                sub_eng.tensor_tensor(
                                    sb[:, q:q + 1, :, :],
                                    e_p[:, g0 + q:g0 + q + 1, :, :],
                                    sb[:, q:q + 1, :, :], Op.subtract)
                                slot = (n_ * n_zg + g0 // zg) * zg + q
                                nc.vector.tensor_tensor_reduce(
                                    sb[:, q:q + 1, :, :], sb[:, q:q + 1, :, :],
                                    sb[:, q:q + 1, :, :], 1.0, 0.0,
                                    Op.mult, Op.add,
                                    loss_acc[:, slot:slot + 1])
                            return
                        if BSQA:
                            sub_eng.tensor_tensor(
                                sb, e_p[:, g0:g0 + zg, :, :], sb, Op.subtract)
                            slot = (n_ * n_zg + g0 // zg) * zg
                            nc.scalar.activation(
                                sb, sb, Act.Square,
                                accum_out=loss_acc[:, slot:slot + 1])
                        else:
                            for q in range(zg):
                                sub_eng.tensor_tensor(
                                    sb[:, q:q + 1, :, :],
                                    e_p[:, g0 + q:g0 + q + 1, :, :],
                                    sb[:, q:q + 1, :, :], Op.subtract)
                                slot = (n_ * n_zg + g0 // zg) * zg + q
                                nc.scalar.activation(
                                    sb[:, q:q + 1, :, :], sb[:, q:q + 1, :, :],
                                    Act.Square,
                                    accum_out=loss_acc[:, slot:slot + 1])

                    def drain_emits(max_z_excl):
                        nonlocal stage_d
                        while emitted[0] < min(nz, max_z_excl):
                            zi = emitted[0]
                            if zi % zg == 0:
                                stage_d = stpool.tile([H, zg, CH, W], dt.bfloat16,
                                                      tag="stg_d", name="stage_d")
                                groups[zi] = (stage_d[:], {})
                            if (zi % (2 * zg) == 0) if UB2 else (zi % zg == 0):
                                ub_cur[0] = pbpool.tile(
                                    [H, 2 * zg, W] if UB2 else [H, zg, W],
                                    dt.float32,
                                                        tag="ub", name="ub")
                            emit_z(zi)
                            emitted[0] += 1
                            if emitted[0] % zg == 0:
                                g0 = emitted[0] - zg
                                close_group(g0)
                                ctx = (g0, t, n, groups)
                                sk = skews or (SKEW_A1, SKEW_A2, SKEW_B)
                                pend.append([tail_a1, ctx, gslot[0] + sk[0]])
                                pend.append([tail_a2, ctx, gslot[0] + sk[1]])
                                if t == "t":
                                    pend.append([tail_b, ctx, gslot[0] + sk[2]])
                            gslot[0] += 1
                            while pend and pend[0][2] <= gslot[0]:
                                fn_, ctx_, _ = pend.pop(0)
                                fn_(*ctx_)

                    return dict(do_diffs=do_diffs, do_square=do_square,
                                do_chansum=do_chansum, drain=drain_emits,
                                produced=set())

            e_p_cur = [None]
            objs = {}

            def get_obj(k):
                if k >= len(passes) or k in objs:
                    return objs.get(k)
                n_, t_ = passes[k]
                if t_ == "p":
                    e_p_cur[0] = eppool.tile(
                        [H, nz, CH, W],
                        dt.bfloat16 if EPBF else dt.float8e4,
                        tag="ep", name="e_p")
                last = (k == len(passes) - 1)
                objs[k] = make_pass(
                    k, n_, t_, e_p_cur[0],
                    skews=(SKL if last else SKP0 if k == 0 else
                           SKP1 if k == 1 else SKP2),
                    ls_eng=LS_LAST if last else LS_MID)
                return objs[k]

            def produce(o, b):
                if b not in o['produced']:
                    o['do_diffs'](b)
                    o['do_square'](b)
                    o['produced'].add(b)

            load_pass(0)
            for k in range(len(passes)):
                o = get_obj(k)
                for b in range(nblk):
                    produce(o, b)
                    # emits needing blocks <= b-1 go to PE BEFORE chansum(b)
                    # so they don't queue behind square(b)
                    if b >= 1:
                        o['drain'](3 * (b - 1) + 1)
                    o['do_chansum'](b)
                    if b == 2:
                        load_pass(k + 1)
                    nxt = get_obj(k + 1) if b >= PF_B else None
                    if b == PF_B and nxt:
                        produce(nxt, 0)
                        nxt['do_chansum'](0)
                        if PF2:
                            produce(nxt, 1)
                            nxt['do_chansum'](1)
                    if b == PF_B + 1 and nxt:
                        produce(nxt, 1)
                        nxt['do_chansum'](1)
                        if PREFILL3:
                            produce(nxt, 2)
                            nxt['do_chansum'](2)
                o['drain'](nz)
            while pend:
                fn_, ctx_, _ = pend.pop(0)
                fn_(*ctx_)

            nc.sync.dma_start(out=out_stats[:], in_=loss_acc[:])

    nc.compile()
    return nc


def _prep_core(vol, z0, nz):
    """vol: (N, D, H, W) f32 -> (img, xh) bf16 W-padded host-side."""
    D = vol.shape[1]
    ns = nz + 6
    nsq = nz + 2
    idx = np.clip(np.arange(z0 - 3, z0 - 3 + ns), 0, D - 1)
    img = vol[:, idx]
    idxq = np.clip(np.arange(z0 - 1, z0 - 1 + nsq), 0, D - 1)
    base = vol[:, idxq]
    hp = np.clip(np.arange(H) + 2, 0, H - 1)
    hm = np.clip(np.arange(H) - 2, 0, H - 1)
    xh = np.stack([base[:, :, hp, :], base[:, :, hm, :]], axis=1)

    def padw(a):
        return np.pad(a, (((0, 0),) * (a.ndim - 1)) + ((3, 3),),
                      mode='edge').astype(BF16)

    img_t = np.ascontiguousarray(padw(img).transpose(0, 2, 1, 3))
    xh_t = np.ascontiguousarray(padw(xh).transpose(0, 3, 1, 2, 4))
    return img_t, xh_t


def _taps_for_core(first, last):
    """[H, 7, 2, H] fp8: 0=(A,A) 1=z0-pair 2=zL-pair 3=zL-wl 4=(I,I)
    5=(A,Z) 6=zL-lone-pair."""
    A = _blur_matrix()
    Z = np.zeros_like(A)
    I = np.eye(H, dtype=np.float32)
    r0 = np.stack([A, A])
    r1 = np.stack([Z, 2 * A]) if first else r0
    r2 = np.stack([A, 2 * A]) if last else r0
    r3 = np.stack([Z, Z]) if last else r0
    r4 = np.stack([I, I])
    r5 = np.stack([A, Z])
    r6 = np.stack([Z, Z]) if last else r5
    taps = np.stack([r0, r1, r2, r3, r4, r5, r6])  # [7, 2, H(k), H(m)]
    taps = taps.transpose(2, 0, 1, 3)              # [H(k), 5, 2, H(m)]
    return np.ascontiguousarray(taps.astype(FP8))


def make_in_maps(p, t, nz=NZ, ncores=NCORES):
    in_maps = []
    for c in range(ncores):
        z0 = c * nz
        img_p, xh_p = _prep_core(p, z0, nz)
        img_t, xh_t = _prep_core(t, z0, nz)
        in_maps.append({
            "img_p": img_p, "xh_p": xh_p,
            "img_t": img_t, "xh_t": xh_t,
            "taps": _taps_for_core(c == 0, c == ncores - 1),
        })
    return in_maps


LAST_RESULTS = None


def kernel(predict, target):
    global LAST_RESULTS
    from concourse import bass_utils

    p = np.ascontiguousarray(np.asarray(predict)[:, 0])
    t = np.ascontiguousarray(np.asarray(target)[:, 0])

    nc = build_bass()
    in_maps = make_in_maps(p, t)

    trace = bool(int(os.environ.get("MIND_TRACE", "0")))
    res = bass_utils.run_bass_kernel_spmd(
        nc, in_maps, core_ids=list(range(NCORES)), trace=trace)
    LAST_RESULTS = res
    total = sum(float(r["out_stats"].astype(np.float64).sum())
                for r in res.results)
    loss = total / TOTAL_COUNT
    return np.array(loss, dtype=np.float32)


if __name__ == "__main__":
    pred = np.load("/root/problem/inp_p.npy")
    targ = np.load("/root/problem/inp_t.npy")
    print("loss:", kernel(pred, targ))
